# revision 19
# baseline (speedup 1.0000x reference)
import time
from contextlib import ExitStack

import numpy as np

BOS_IDX = 0
NCORES = 8
L = 128
M_SLOTS = 36
QT = 9

LAST = {}


def _bf16(x):
    import ml_dtypes
    return np.asarray(x).astype(ml_dtypes.bfloat16)


def _perron_c(transitions):
    W64 = np.exp(transitions.astype(np.float64))
    v = np.ones(L)
    for _ in range(200):
        v = W64 @ v
        v /= np.linalg.norm(v)
    lam1 = float(v @ W64 @ v) / float(v @ v)
    return float(np.log(lam1) + 0.5)


def _host_prep(logits, transitions, lens, M=M_SLOTS):
    B, S, Lc = logits.shape
    assert Lc == L
    lens = np.asarray(lens).astype(np.int64)

    c = _perron_c(transitions)
    Wp64 = np.exp(transitions.astype(np.float64) - c)
    wf = _bf16(np.ascontiguousarray(Wp64.T))
    wb = _bf16(np.ascontiguousarray(Wp64))
    z_pad64 = np.linalg.solve(Wp64.T, np.ones(L))

    elog = np.exp(logits.astype(np.float32))

    int_jobs = []
    tail_jobs = []
    seq_k = np.zeros(B, np.int64)
    for b in range(B):
        l = int(lens[b])
        k = max(1, -(-l // M))
        seq_k[b] = k
        cuts = [min(i * M, l) for i in range(k + 1)]
        for j in range(k - 1):
            int_jobs.append((b, cuts[j], cuts[j + 1]))
        tail_jobs.append((cuts[k] - cuts[k - 1], b, cuts[k - 1], cuts[k]))

    NI = -(-len(int_jobs) // NCORES)
    NI += NI % 2
    int_rank = [[None] * NI for _ in range(NCORES)]
    for i, job in enumerate(int_jobs):
        int_rank[i % NCORES][i // NCORES] = job

    tail_jobs.sort(key=lambda j: -j[0])
    NT = -(-len(tail_jobs) // NCORES)
    tail_rank = [[None] * NT for _ in range(NCORES)]
    for i, job in enumerate(tail_jobs):
        tail_rank[i % NCORES][i // NCORES] = job
    BL = np.zeros(NT, np.int64)
    for i in range(NT):
        mx = max((tail_rank[m][i][0] if tail_rank[m][i] else 1)
                 for m in range(NCORES))
        BL[i] = min(M, -(-mx // QT) * QT)

    NB = NI + NT
    NB += NB % 2
    NSLOT = M

    nb = np.zeros(NSLOT, np.int64)
    for s in range(NSLOT):
        at = np.nonzero(BL > s)[0]
        nb[s] = NI + ((at.max() + 1) if at.size else 0)

    bwd_caps = {}
    for i in range(NT):
        bwd_caps.setdefault(int(BL[i] - 1), []).append(NI + i)
    if NI > 0:
        bwd_caps.setdefault(NSLOT - 1, []).extend(range(NI))

    def runs(ks):
        out = []
        for kk in sorted(ks):
            if out and out[-1][1] == kk:
                out[-1] = (out[-1][0], kk + 1)
            else:
                out.append((kk, kk + 1))
        return out

    bwd_caps = {s: runs(v) for s, v in bwd_caps.items()}

    place = {}
    streams = []
    z_pad32 = z_pad64.astype(np.float32)
    for m in range(NCORES):
        eb = np.ones((NSLOT, L, NB), np.float32)
        for i in range(NI):
            job = int_rank[m][i]
            if job is None:
                continue
            b, t0, t1 = job
            eb[:, :, i] = elog[b, t0:t1, :][::-1]
            place[(b, 'i', t0)] = (m, i)
        for i in range(NT):
            job = tail_rank[m][i]
            if job is None:
                continue
            ln_, b, t0, t1 = job
            pad = int(BL[i]) - ln_
            if pad:
                eb[:pad, :, NI + i] = z_pad32[None, :]
            eb[pad:BL[i], :, NI + i] = elog[b, t0:t1, :][::-1]
            place[(b, 't', t0)] = (m, NI + i)
        streams.append(_bf16(np.ascontiguousarray(
            eb.transpose(1, 0, 2).reshape(L, NSLOT * NB))))

    return dict(c=c, wf=wf, wb=wb, streams=streams, NSLOT=NSLOT, NI=NI,
                NB=NB, nb=nb, bwd_caps=bwd_caps, place=place, lens=lens,
                seq_k=seq_k, M=M)


def _host_combine(prep, capf, capb):
    c = prep["c"]
    lens = prep["lens"]
    seq_k = prep["seq_k"]
    place = prep["place"]
    M = prep["M"]
    B = len(lens)
    ones = np.ones(L, np.float64)

    def vecA(b, t0):
        m, i = place[(b, 'i', t0)]
        return capf[m][:, i].astype(np.float64)

    def vecB(b, t0):
        m, i = place[(b, 'i', t0)]
        return capb[m][:, i].astype(np.float64)

    def vecH(b, t0):
        m, i = place[(b, 't', t0)]
        return capb[m][:, i].astype(np.float64)

    logZ = np.empty(B, np.float64)
    for b in range(B):
        l = int(lens[b])
        k = int(seq_k[b])
        cuts = [min(i * M, l) for i in range(k + 1)]
        H = vecH(b, cuts[k - 1])
        if k == 1:
            logZ[b] = np.log(H[BOS_IDX]) + c * l
            continue
        A_last = vecA(b, cuts[k - 2])
        lz = np.log(H @ A_last)
        for j in range(2, k):
            Bj = vecB(b, cuts[j - 1])
            Aprev = vecA(b, cuts[j - 2])
            lz += np.log(Bj @ Aprev) - np.log(Bj @ ones)
        B1 = vecB(b, cuts[0])
        lz += np.log(B1[BOS_IDX]) - np.log(B1 @ ones)
        logZ[b] = lz + c * l
    return logZ.astype(np.float32)


def _default_splits(NI, NB):
    return ((NI, 'D'),), ((NB, 'D'),)


def _build_bass(NSLOT, NI, NB, nb, bwd_caps, repeat=1, timing=False,
                noserial=False, caps=True, fsplit=None, bsplit=None):
    import concourse.bacc as bacc
    import concourse.mybir as mybir
    import concourse.tile as tile

    f32 = mybir.dt.float32
    bf16 = mybir.dt.bfloat16
    if fsplit is None or bsplit is None:
        fsplit, bsplit = _default_splits(NI, NB)
    assert sum(w for w, _ in fsplit) == NI
    assert sum(w for w, _ in bsplit) == NB
    nc = bacc.Bacc("TRN2", target_bir_lowering=False, debug=False,
                   num_devices=NCORES)

    stream_kind = "Internal" if timing else "ExternalInput"
    eb_d = nc.dram_tensor("eb", [L, NSLOT * NB], bf16, kind=stream_kind).ap()
    wf_d = nc.dram_tensor("wf", [L, L], bf16, kind="ExternalInput").ap()
    wb_d = nc.dram_tensor("wb", [L, L], bf16, kind="ExternalInput").ap()
    capf_d = nc.dram_tensor("capf", [L, NI], f32, kind="ExternalOutput").ap()
    capb_d = nc.dram_tensor("capb", [L, NB], f32, kind="ExternalOutput").ap()

    CH = max(4, (NSLOT + 3) // 4)
    nch = -(-NSLOT // CH)
    lo = [(i * CH, min(NSLOT, (i + 1) * CH)) for i in range(nch)]
    order = []
    a, bidx = 0, nch - 1
    while a <= bidx:
        if a != bidx:
            order.extend([lo[bidx], lo[a]])
        else:
            order.append(lo[a])
        a += 1
        bidx -= 1

    fchains = []
    off = 0
    for w, path in fsplit:
        fchains.append((off, off + w, path))
        off += w
    bchains = []
    off = 0
    for w, path in bsplit:
        bchains.append((off, off + w, path))
        off += w

    with tile.TileContext(nc) as tc, ExitStack() as ctx:
        cpool = ctx.enter_context(tc.tile_pool(name="const", bufs=1))
        spool = ctx.enter_context(tc.tile_pool(name="state", bufs=3))
        ypool = ctx.enter_context(tc.tile_pool(name="evac", bufs=2))
        strm = ctx.enter_context(tc.tile_pool(name="stream", bufs=2))
        psf = [ctx.enter_context(
            tc.tile_pool(name=f"psf{g}", bufs=1, space="PSUM"))
            for g in range(len(fchains))]
        psb = [ctx.enter_context(
            tc.tile_pool(name=f"psb{g}", bufs=1, space="PSUM"))
            for g in range(len(bchains))]

        wf_t = cpool.tile([L, L], bf16, tag="wf")
        nc.sync.dma_start(wf_t[:], wf_d[:])
        wb_t = cpool.tile([L, L], bf16, tag="wb")
        nc.sync.dma_start(wb_t[:], wb_d[:])

        capF = cpool.tile([L, NI], f32, tag="capF")
        nc.vector.memset(capF[:], 1.0)
        capB = cpool.tile([L, NB], f32, tag="capB")
        nc.vector.memset(capB[:], 1.0)

        serial = timing and not noserial
        loop = timing
        if serial:
            pcar = [cpool.tile([L, 8], bf16, tag=f"pcar{g}", name=f"pcar{g}")
                    for g in range(len(fchains))]
            hcar = [cpool.tile([L, 8], bf16, tag=f"hcar{g}", name=f"hcar{g}")
                    for g in range(len(bchains))]
            for t in pcar + hcar:
                nc.vector.memset(t[:], 1.0)

        def one_pass():
            S = strm.tile([L, NSLOT * NB], bf16, tag="S")
            for s0, s1 in order:
                nc.sync.dma_start(S[:, s0 * NB:s1 * NB],
                                  eb_d[:, s0 * NB:s1 * NB])

            def ef_ap(s, c0, c1):
                off = (NSLOT - 1 - s) * NB
                return S[:, off + c0:off + c1]

            def eb_ap(s, c0, c1):
                off = s * NB
                return S[:, off + c0:off + c1]

            ps = []
            for g, (c0, c1, _) in enumerate(fchains):
                p = spool.tile([L, c1 - c0], bf16, tag=f"p{g}")
                nc.vector.memset(p[:], 1.0)
                if serial:
                    nc.scalar.copy(p[:, :8], pcar[g][:])
                ps.append(p)

            vbs = [None] * len(bchains)
            if serial:
                for g, (c0, c1, _) in enumerate(bchains):
                    aw = max(0, min(c1, int(nb[0])) - c0)
                    if aw <= 0:
                        continue
                    vb = spool.tile([L, aw], bf16, tag=f"vb{g}")
                    nc.vector.tensor_copy(vb[:], eb_ap(0, c0, c0 + aw))
                    nc.vector.tensor_mul(vb[:, :8], hcar[g][:],
                                         eb_ap(0, c0, c0 + 8))
                    vbs[g] = vb

            qbs = [None] * len(bchains)
            for s in range(NSLOT):
                last = s == NSLOT - 1
                for g, (c0, c1, path) in enumerate(fchains):
                    w = c1 - c0
                    qf = psf[g].tile([L, w], f32, tag=f"qf{g}")
                    nc.tensor.matmul(qf[:], wf_t[:], ps[g][:])
                    if last:
                        nc.vector.tensor_mul(capF[:, c0:c1], qf[:],
                                             ef_ap(s, c0, c1))
                    elif path == 'Y':
                        qf_sb = ypool.tile([L, w], bf16, tag=f"qfs{g}")
                        nc.scalar.copy(qf_sb[:], qf[:])
                        ps[g] = spool.tile([L, w], bf16, tag=f"p{g}", name=f"p{g}")
                        nc.vector.tensor_mul(ps[g][:], qf_sb[:],
                                             ef_ap(s, c0, c1))
                    else:
                        ps[g] = spool.tile([L, w], bf16, tag=f"p{g}", name=f"p{g}")
                        nc.vector.tensor_mul(ps[g][:], qf[:],
                                             ef_ap(s, c0, c1))

                for g, (c0, c1, path) in enumerate(bchains):
                    aw = max(0, min(c1, int(nb[s])) - c0)
                    if aw <= 0:
                        continue
                    qb = psb[g].tile([L, aw], f32, tag=f"qb{g}")
                    nc.tensor.matmul(
                        qb[:], wb_t[:],
                        vbs[g][:, :aw] if vbs[g] is not None
                        else eb_ap(s, c0, c0 + aw))
                    qbs[g] = qb
                    if caps:
                        for lo_, hi_ in bwd_caps.get(s, []):
                            lo2, hi2 = max(lo_, c0), min(hi_, c1)
                            if lo2 < hi2:
                                nc.scalar.copy(capB[:, lo2:hi2],
                                               qb[:, lo2 - c0:hi2 - c0])
                    if last:
                        continue
                    nxt = max(0, min(c1, int(nb[s + 1])) - c0)
                    if nxt <= 0:
                        vbs[g] = None
                        continue
                    if path == 'Y':
                        hb_sb = ypool.tile([L, nxt], bf16, tag=f"hbs{g}")
                        nc.scalar.copy(hb_sb[:], qb[:, :nxt])
                        vbs[g] = spool.tile([L, nxt], bf16, tag=f"vb{g}", name=f"vb{g}")
                        nc.vector.tensor_mul(vbs[g][:], hb_sb[:],
                                             eb_ap(s + 1, c0, c0 + nxt))
                    else:
                        vbs[g] = spool.tile([L, nxt], bf16, tag=f"vb{g}", name=f"vb{g}")
                        nc.vector.tensor_mul(vbs[g][:], qb[:, :nxt],
                                             eb_ap(s + 1, c0, c0 + nxt))

            if serial:
                for g, (c0, c1, _) in enumerate(fchains):
                    nc.scalar.copy(pcar[g][:], capF[:, c0:c0 + 8])
                for g in range(len(bchains)):
                    if qbs[g] is not None:
                        nc.scalar.copy(hcar[g][:], qbs[g][:, :8])

        if loop:
            with tc.For_i(0, repeat):
                one_pass()
        else:
            assert repeat == 1
            one_pass()

        nc.sync.dma_start(capf_d[:], capF[:])
        nc.sync.dma_start(capb_d[:], capB[:])

    nc.compile()
    return nc


def kernel(logits, transitions, lens):
    from concourse.bass_utils import run_bass_kernel_spmd

    logits = np.asarray(logits, dtype=np.float32)
    transitions = np.asarray(transitions, dtype=np.float32)
    lens_in = np.asarray(lens)

    prep = _host_prep(logits, transitions, lens_in, M=M_SLOTS)

    t0 = time.time()
    nc = _build_bass(prep["NSLOT"], prep["NI"], prep["NB"], prep["nb"],
                     prep["bwd_caps"])
    t1 = time.time()

    in_maps = [{"eb": prep["streams"][m], "wf": prep["wf"], "wb": prep["wb"]}
               for m in range(NCORES)]
    try:
        r = run_bass_kernel_spmd(nc, in_maps, core_ids=list(range(NCORES)))
    except Exception:
        time.sleep(10)
        r = run_bass_kernel_spmd(nc, in_maps, core_ids=list(range(NCORES)))
    t2 = time.time()

    capf = [r.results[m]["capf"] for m in range(NCORES)]
    capb = [r.results[m]["capb"] for m in range(NCORES)]
    out = _host_combine(prep, capf, capb)

    LAST.clear()
    LAST.update(build_s=t1 - t0, run_s=t2 - t1, results=r,
                exec_time_ns=r.exec_time_ns, nslot=prep["NSLOT"],
                cols=prep["NB"])
    return out


if __name__ == "__main__":
    rng = np.random.default_rng(0)
    B, S = 512, 512
    logits = rng.standard_normal((B, S, L), dtype=np.float32)
    lens = rng.integers(1, S + 1, size=B).astype(np.int64)
    transitions = rng.standard_normal((L, L)).astype(np.float32)
    out = kernel(logits=logits, transitions=transitions, lens=lens)
    print("out[:8] =", out[:8])
    print("timings:", {k: LAST[k] for k in ("build_s", "run_s", "nslot")})


# revision 25
# speedup vs baseline: 1.0097x; 1.0097x over previous
import time
from contextlib import ExitStack

import numpy as np

BOS_IDX = 0
NCORES = 8
L = 128
M_SLOTS = 36
QT = 9

LAST = {}


def _bf16(x):
    import ml_dtypes
    return np.asarray(x).astype(ml_dtypes.bfloat16)


def _perron_c(transitions):
    W64 = np.exp(transitions.astype(np.float64))
    v = np.ones(L)
    for _ in range(200):
        v = W64 @ v
        v /= np.linalg.norm(v)
    lam1 = float(v @ W64 @ v) / float(v @ v)
    return float(np.log(lam1) + 0.5)


def _host_prep(logits, transitions, lens, M=M_SLOTS):
    B, S, Lc = logits.shape
    assert Lc == L
    lens = np.asarray(lens).astype(np.int64)

    c = _perron_c(transitions)
    Wp64 = np.exp(transitions.astype(np.float64) - c)
    wf = _bf16(np.ascontiguousarray(Wp64.T))
    wb = _bf16(np.ascontiguousarray(Wp64))
    z_pad64 = np.linalg.solve(Wp64.T, np.ones(L))

    elog = np.exp(logits.astype(np.float32))

    int_jobs = []
    tail_jobs = []
    seq_k = np.zeros(B, np.int64)
    for b in range(B):
        l = int(lens[b])
        k = max(1, -(-l // M))
        seq_k[b] = k
        cuts = [min(i * M, l) for i in range(k + 1)]
        for j in range(k - 1):
            int_jobs.append((b, cuts[j], cuts[j + 1]))
        tail_jobs.append((cuts[k] - cuts[k - 1], b, cuts[k - 1], cuts[k]))

    NI = -(-len(int_jobs) // NCORES)
    NI += NI % 2
    int_rank = [[None] * NI for _ in range(NCORES)]
    for i, job in enumerate(int_jobs):
        int_rank[i % NCORES][i // NCORES] = job

    tail_jobs.sort(key=lambda j: -j[0])
    NT = -(-len(tail_jobs) // NCORES)
    tail_rank = [[None] * NT for _ in range(NCORES)]
    for i, job in enumerate(tail_jobs):
        tail_rank[i % NCORES][i // NCORES] = job
    BL = np.zeros(NT, np.int64)
    for i in range(NT):
        mx = max((tail_rank[m][i][0] if tail_rank[m][i] else 1)
                 for m in range(NCORES))
        BL[i] = min(M, -(-mx // QT) * QT)

    NB = NI + NT
    NB += NB % 2
    NSLOT = M

    nb = np.zeros(NSLOT, np.int64)
    for s in range(NSLOT):
        at = np.nonzero(BL > s)[0]
        nb[s] = NI + ((at.max() + 1) if at.size else 0)

    bwd_caps = {}
    for i in range(NT):
        bwd_caps.setdefault(int(BL[i] - 1), []).append(NI + i)
    if NI > 0:
        bwd_caps.setdefault(NSLOT - 1, []).extend(range(NI))

    def runs(ks):
        out = []
        for kk in sorted(ks):
            if out and out[-1][1] == kk:
                out[-1] = (out[-1][0], kk + 1)
            else:
                out.append((kk, kk + 1))
        return out

    bwd_caps = {s: runs(v) for s, v in bwd_caps.items()}

    place = {}
    streams = []
    z_pad32 = z_pad64.astype(np.float32)
    for m in range(NCORES):
        eb = np.ones((NSLOT, L, NB), np.float32)
        for i in range(NI):
            job = int_rank[m][i]
            if job is None:
                continue
            b, t0, t1 = job
            eb[:, :, i] = elog[b, t0:t1, :][::-1]
            place[(b, 'i', t0)] = (m, i)
        for i in range(NT):
            job = tail_rank[m][i]
            if job is None:
                continue
            ln_, b, t0, t1 = job
            pad = int(BL[i]) - ln_
            if pad:
                eb[:pad, :, NI + i] = z_pad32[None, :]
            eb[pad:BL[i], :, NI + i] = elog[b, t0:t1, :][::-1]
            place[(b, 't', t0)] = (m, NI + i)
        streams.append(_bf16(np.ascontiguousarray(
            eb.transpose(1, 0, 2).reshape(L, NSLOT * NB))))

    return dict(c=c, wf=wf, wb=wb, streams=streams, NSLOT=NSLOT, NI=NI,
                NB=NB, nb=nb, bwd_caps=bwd_caps, place=place, lens=lens,
                seq_k=seq_k, M=M)


def _host_combine(prep, capf, capb):
    c = prep["c"]
    lens = prep["lens"]
    seq_k = prep["seq_k"]
    place = prep["place"]
    M = prep["M"]
    B = len(lens)
    ones = np.ones(L, np.float64)

    def vecA(b, t0):
        m, i = place[(b, 'i', t0)]
        return capf[m][:, i].astype(np.float64)

    def vecB(b, t0):
        m, i = place[(b, 'i', t0)]
        return capb[m][:, i].astype(np.float64)

    def vecH(b, t0):
        m, i = place[(b, 't', t0)]
        return capb[m][:, i].astype(np.float64)

    logZ = np.empty(B, np.float64)
    for b in range(B):
        l = int(lens[b])
        k = int(seq_k[b])
        cuts = [min(i * M, l) for i in range(k + 1)]
        H = vecH(b, cuts[k - 1])
        if k == 1:
            logZ[b] = np.log(H[BOS_IDX]) + c * l
            continue
        A_last = vecA(b, cuts[k - 2])
        lz = np.log(H @ A_last)
        for j in range(2, k):
            Bj = vecB(b, cuts[j - 1])
            Aprev = vecA(b, cuts[j - 2])
            lz += np.log(Bj @ Aprev) - np.log(Bj @ ones)
        B1 = vecB(b, cuts[0])
        lz += np.log(B1[BOS_IDX]) - np.log(B1 @ ones)
        logZ[b] = lz + c * l
    return logZ.astype(np.float32)


def _default_splits(NI, NB):
    return ((NI, 'D'),), ((NB, 'D'),)


def _build_bass_merged(NSLOT, NI, NB, nb, bwd_caps, repeat=1, timing=False,
                       noserial=False, caps=True):
    import concourse.bacc as bacc
    import concourse.mybir as mybir
    import concourse.tile as tile

    f32 = mybir.dt.float32
    bf16 = mybir.dt.bfloat16
    nc = bacc.Bacc("TRN2", target_bir_lowering=False, debug=False,
                   num_devices=NCORES)

    stream_kind = "Internal" if timing else "ExternalInput"
    eb_d = nc.dram_tensor("eb", [L, NSLOT * NB], bf16, kind=stream_kind).ap()
    wf_d = nc.dram_tensor("wf", [L, L], bf16, kind="ExternalInput").ap()
    wb_d = nc.dram_tensor("wb", [L, L], bf16, kind="ExternalInput").ap()
    capf_d = nc.dram_tensor("capf", [L, NI], f32, kind="ExternalOutput").ap()
    capb_d = nc.dram_tensor("capb", [L, NB], f32, kind="ExternalOutput").ap()

    CH = max(4, (NSLOT + 3) // 4)
    nch = -(-NSLOT // CH)
    lo = [(i * CH, min(NSLOT, (i + 1) * CH)) for i in range(nch)]
    order = []
    a, bidx = 0, nch - 1
    while a <= bidx:
        if a != bidx:
            order.extend([lo[bidx], lo[a]])
        else:
            order.append(lo[a])
        a += 1
        bidx -= 1

    HALF = 512

    with tile.TileContext(nc) as tc, ExitStack() as ctx:
        cpool = ctx.enter_context(tc.tile_pool(name="const", bufs=1))
        spool = ctx.enter_context(tc.tile_pool(name="state", bufs=3))
        strm = ctx.enter_context(tc.tile_pool(name="stream", bufs=2))
        pspool = ctx.enter_context(tc.tile_pool(name="ps", bufs=1,
                                                space="PSUM"))

        wf_t = cpool.tile([L, L], bf16, tag="wf")
        nc.sync.dma_start(wf_t[:], wf_d[:])
        wb_t = cpool.tile([L, L], bf16, tag="wb")
        nc.sync.dma_start(wb_t[:], wb_d[:])

        capF = cpool.tile([L, NI], f32, tag="capF")
        nc.vector.memset(capF[:], 1.0)
        capB = cpool.tile([L, NB], f32, tag="capB")
        nc.vector.memset(capB[:], 1.0)

        pones = cpool.tile([L, NI], bf16, tag="pones")
        nc.vector.memset(pones[:], 1.0)

        qpair = pspool.tile([L, 2 * HALF], f32, tag="qpair")
        nc.vector.memset(qpair[:], 1.0)

        serial = timing and not noserial
        loop = timing
        if serial:
            hcar = cpool.tile([L, 8], bf16, tag="hcar")
            nc.vector.memset(hcar[:], 1.0)

        def one_pass():
            S = strm.tile([L, NSLOT * NB], bf16, tag="S")
            for s0, s1 in order:
                nc.sync.dma_start(S[:, s0 * NB:s1 * NB],
                                  eb_d[:, s0 * NB:s1 * NB])
            S3 = S[:].rearrange("p (t c) -> p t c", c=NB)
            Q3 = qpair[:].rearrange("p (a c) -> p a c", c=HALF)

            p = pones[:]
            vb = None
            if serial:
                n0 = int(nb[0])
                vbi = spool.tile([L, n0], bf16, tag="vbi")
                nc.vector.tensor_copy(vbi[:], S3[:, 0, :n0])
                nc.vector.tensor_mul(vbi[:, :8], hcar[:], S3[:, 0, :8])
                vb = vbi[:]

            qb_last = None
            qb_blk = 0
            for s in range(NSLOT):
                m = NSLOT - 1 - s
                n = s + 1
                nbs = int(nb[s])
                last = s == NSLOT - 1
                merged = (not last) and m != n
                if merged:
                    blk_f = 0 if m < n else 1
                else:
                    blk_f = 0
                blk_b = 1 - blk_f

                qf = qpair[:, blk_f * HALF:blk_f * HALF + NI]
                nc.tensor.matmul(qf, wf_t[:], p)
                qb = qpair[:, blk_b * HALF:blk_b * HALF + nbs]
                nc.tensor.matmul(qb, wb_t[:],
                                 vb if vb is not None else S3[:, s, :nbs])
                qb_last, qb_blk = qb, blk_b

                if caps:
                    for lo_, hi_ in bwd_caps.get(s, []):
                        nc.scalar.copy(
                            capB[:, lo_:hi_],
                            qpair[:, blk_b * HALF + lo_:blk_b * HALF + hi_])

                if last:
                    nc.vector.tensor_mul(capF[:], qf, S3[:, m, :NI])
                    break
                nxt = int(nb[n])
                if merged:
                    w = max(NI, nxt)
                    w += w & 1
                    st = spool.tile([L, 2 * HALF], bf16, tag="st")
                    st3 = st[:].rearrange("p (a c) -> p a c", c=HALF)
                    slo = min(m, n)
                    D = abs(m - n)
                    nc.vector.tensor_mul(
                        st3[:, :, :w], Q3[:, :, :w],
                        S3[:, slo:slo + D + 1:D, :w])
                    p = st[:, blk_f * HALF:blk_f * HALF + NI]
                    vb = st[:, blk_b * HALF:blk_b * HALF + nxt]
                else:
                    pn = spool.tile([L, NI], bf16, tag="pn")
                    nc.vector.tensor_mul(pn[:], qf, S3[:, m, :NI])
                    vbn = spool.tile([L, nxt], bf16, tag="vbn")
                    nc.vector.tensor_mul(vbn[:], qpair[:, blk_b * HALF:
                                                       blk_b * HALF + nxt],
                                         S3[:, n, :nxt])
                    p = pn[:]
                    vb = vbn[:]

            if serial:
                nc.scalar.copy(pones[:, :8], capF[:, :8])
                nc.scalar.copy(hcar[:], qpair[:, qb_blk * HALF:
                                              qb_blk * HALF + 8])

        if loop:
            with tc.For_i(0, repeat):
                one_pass()
        else:
            assert repeat == 1
            one_pass()

        nc.sync.dma_start(capf_d[:], capF[:])
        nc.sync.dma_start(capb_d[:], capB[:])

    nc.compile()
    return nc


def _build_bass(NSLOT, NI, NB, nb, bwd_caps, repeat=1, timing=False,
                noserial=False, caps=True, fsplit=None, bsplit=None):
    import concourse.bacc as bacc
    import concourse.mybir as mybir
    import concourse.tile as tile

    f32 = mybir.dt.float32
    bf16 = mybir.dt.bfloat16
    if fsplit is None or bsplit is None:
        fsplit, bsplit = _default_splits(NI, NB)
    assert sum(w for w, _ in fsplit) == NI
    assert sum(w for w, _ in bsplit) == NB
    nc = bacc.Bacc("TRN2", target_bir_lowering=False, debug=False,
                   num_devices=NCORES)

    stream_kind = "Internal" if timing else "ExternalInput"
    eb_d = nc.dram_tensor("eb", [L, NSLOT * NB], bf16, kind=stream_kind).ap()
    wf_d = nc.dram_tensor("wf", [L, L], bf16, kind="ExternalInput").ap()
    wb_d = nc.dram_tensor("wb", [L, L], bf16, kind="ExternalInput").ap()
    capf_d = nc.dram_tensor("capf", [L, NI], f32, kind="ExternalOutput").ap()
    capb_d = nc.dram_tensor("capb", [L, NB], f32, kind="ExternalOutput").ap()

    CH = max(4, (NSLOT + 3) // 4)
    nch = -(-NSLOT // CH)
    lo = [(i * CH, min(NSLOT, (i + 1) * CH)) for i in range(nch)]
    order = []
    a, bidx = 0, nch - 1
    while a <= bidx:
        if a != bidx:
            order.extend([lo[bidx], lo[a]])
        else:
            order.append(lo[a])
        a += 1
        bidx -= 1

    fchains = []
    off = 0
    for w, path in fsplit:
        fchains.append((off, off + w, path))
        off += w
    bchains = []
    off = 0
    for w, path in bsplit:
        bchains.append((off, off + w, path))
        off += w

    with tile.TileContext(nc) as tc, ExitStack() as ctx:
        cpool = ctx.enter_context(tc.tile_pool(name="const", bufs=1))
        spool = ctx.enter_context(tc.tile_pool(name="state", bufs=3))
        ypool = ctx.enter_context(tc.tile_pool(name="evac", bufs=2))
        strm = ctx.enter_context(tc.tile_pool(name="stream", bufs=2))
        psf = [ctx.enter_context(
            tc.tile_pool(name=f"psf{g}", bufs=1, space="PSUM"))
            for g in range(len(fchains))]
        psb = [ctx.enter_context(
            tc.tile_pool(name=f"psb{g}", bufs=1, space="PSUM"))
            for g in range(len(bchains))]

        wf_t = cpool.tile([L, L], bf16, tag="wf")
        nc.sync.dma_start(wf_t[:], wf_d[:])
        wb_t = cpool.tile([L, L], bf16, tag="wb")
        nc.sync.dma_start(wb_t[:], wb_d[:])

        capF = cpool.tile([L, NI], f32, tag="capF")
        nc.vector.memset(capF[:], 1.0)
        capB = cpool.tile([L, NB], f32, tag="capB")
        nc.vector.memset(capB[:], 1.0)

        pone = [cpool.tile([L, c1 - c0], bf16, tag=f"pone{g}",
                           name=f"pone{g}")
                for g, (c0, c1, _) in enumerate(fchains)]
        for t in pone:
            nc.vector.memset(t[:], 1.0)

        serial = timing and not noserial
        loop = timing
        if serial:
            hcar = [cpool.tile([L, 8], bf16, tag=f"hcar{g}", name=f"hcar{g}")
                    for g in range(len(bchains))]
            for t in hcar:
                nc.vector.memset(t[:], 1.0)

        def one_pass():
            S = strm.tile([L, NSLOT * NB], bf16, tag="S")
            for s0, s1 in order:
                nc.sync.dma_start(S[:, s0 * NB:s1 * NB],
                                  eb_d[:, s0 * NB:s1 * NB])

            def ef_ap(s, c0, c1):
                off = (NSLOT - 1 - s) * NB
                return S[:, off + c0:off + c1]

            def eb_ap(s, c0, c1):
                off = s * NB
                return S[:, off + c0:off + c1]

            ps = [pone[g] for g in range(len(fchains))]

            vbs = [None] * len(bchains)
            if serial:
                for g, (c0, c1, _) in enumerate(bchains):
                    aw = max(0, min(c1, int(nb[0])) - c0)
                    if aw <= 0:
                        continue
                    vb = spool.tile([L, aw], bf16, tag=f"vb{g}")
                    nc.vector.tensor_copy(vb[:], eb_ap(0, c0, c0 + aw))
                    nc.vector.tensor_mul(vb[:, :8], hcar[g][:],
                                         eb_ap(0, c0, c0 + 8))
                    vbs[g] = vb

            qbs = [None] * len(bchains)
            for s in range(NSLOT):
                last = s == NSLOT - 1
                for g, (c0, c1, path) in enumerate(fchains):
                    w = c1 - c0
                    qf = psf[g].tile([L, w], f32, tag=f"qf{g}")
                    nc.tensor.matmul(qf[:], wf_t[:], ps[g][:])
                    if last:
                        nc.vector.tensor_mul(capF[:, c0:c1], qf[:],
                                             ef_ap(s, c0, c1))
                    elif path == 'Y':
                        qf_sb = ypool.tile([L, w], bf16, tag=f"qfs{g}")
                        nc.scalar.copy(qf_sb[:], qf[:])
                        ps[g] = spool.tile([L, w], bf16, tag=f"p{g}", name=f"p{g}")
                        nc.vector.tensor_mul(ps[g][:], qf_sb[:],
                                             ef_ap(s, c0, c1))
                    else:
                        ps[g] = spool.tile([L, w], bf16, tag=f"p{g}", name=f"p{g}")
                        nc.vector.tensor_mul(ps[g][:], qf[:],
                                             ef_ap(s, c0, c1))

                for g, (c0, c1, path) in enumerate(bchains):
                    aw = max(0, min(c1, int(nb[s])) - c0)
                    if aw <= 0:
                        continue
                    qb = psb[g].tile([L, aw], f32, tag=f"qb{g}")
                    nc.tensor.matmul(
                        qb[:], wb_t[:],
                        vbs[g][:, :aw] if vbs[g] is not None
                        else eb_ap(s, c0, c0 + aw))
                    qbs[g] = qb
                    if caps:
                        for lo_, hi_ in bwd_caps.get(s, []):
                            lo2, hi2 = max(lo_, c0), min(hi_, c1)
                            if lo2 < hi2:
                                nc.scalar.copy(capB[:, lo2:hi2],
                                               qb[:, lo2 - c0:hi2 - c0])
                    if last:
                        continue
                    nxt = max(0, min(c1, int(nb[s + 1])) - c0)
                    if nxt <= 0:
                        vbs[g] = None
                        continue
                    if path == 'Y':
                        hb_sb = ypool.tile([L, nxt], bf16, tag=f"hbs{g}")
                        nc.scalar.copy(hb_sb[:], qb[:, :nxt])
                        vbs[g] = spool.tile([L, nxt], bf16, tag=f"vb{g}", name=f"vb{g}")
                        nc.vector.tensor_mul(vbs[g][:], hb_sb[:],
                                             eb_ap(s + 1, c0, c0 + nxt))
                    else:
                        vbs[g] = spool.tile([L, nxt], bf16, tag=f"vb{g}", name=f"vb{g}")
                        nc.vector.tensor_mul(vbs[g][:], qb[:, :nxt],
                                             eb_ap(s + 1, c0, c0 + nxt))

            if serial:
                for g, (c0, c1, _) in enumerate(fchains):
                    nc.scalar.copy(pone[g][:, :8], capF[:, c0:c0 + 8])
                for g in range(len(bchains)):
                    if qbs[g] is not None:
                        nc.scalar.copy(hcar[g][:], qbs[g][:, :8])

        if loop:
            with tc.For_i(0, repeat):
                one_pass()
        else:
            assert repeat == 1
            one_pass()

        nc.sync.dma_start(capf_d[:], capF[:])
        nc.sync.dma_start(capb_d[:], capB[:])

    nc.compile()
    return nc


def kernel(logits, transitions, lens):
    from concourse.bass_utils import run_bass_kernel_spmd

    logits = np.asarray(logits, dtype=np.float32)
    transitions = np.asarray(transitions, dtype=np.float32)
    lens_in = np.asarray(lens)

    prep = _host_prep(logits, transitions, lens_in, M=M_SLOTS)

    t0 = time.time()
    nc = _build_bass(prep["NSLOT"], prep["NI"], prep["NB"], prep["nb"],
                     prep["bwd_caps"])
    t1 = time.time()

    in_maps = [{"eb": prep["streams"][m], "wf": prep["wf"], "wb": prep["wb"]}
               for m in range(NCORES)]
    try:
        r = run_bass_kernel_spmd(nc, in_maps, core_ids=list(range(NCORES)))
    except Exception:
        time.sleep(10)
        r = run_bass_kernel_spmd(nc, in_maps, core_ids=list(range(NCORES)))
    t2 = time.time()

    capf = [r.results[m]["capf"] for m in range(NCORES)]
    capb = [r.results[m]["capb"] for m in range(NCORES)]
    out = _host_combine(prep, capf, capb)

    LAST.clear()
    LAST.update(build_s=t1 - t0, run_s=t2 - t1, results=r,
                exec_time_ns=r.exec_time_ns, nslot=prep["NSLOT"],
                cols=prep["NB"])
    return out


if __name__ == "__main__":
    rng = np.random.default_rng(0)
    B, S = 512, 512
    logits = rng.standard_normal((B, S, L), dtype=np.float32)
    lens = rng.integers(1, S + 1, size=B).astype(np.int64)
    transitions = rng.standard_normal((L, L)).astype(np.float32)
    out = kernel(logits=logits, transitions=transitions, lens=lens)
    print("out[:8] =", out[:8])
    print("timings:", {k: LAST[k] for k in ("build_s", "run_s", "nslot")})
